# revision 9
# baseline (speedup 1.0000x reference)
"""Multi-head causal self-attention on 8 Trainium2 NeuronCores.

Problem: B=256, T=256, E=384, H=6, D=64 (fp32).
Strategy: pure data parallelism over the batch dim — each of the 8 cores
processes 32 batches end-to-end (QKV projections, causal softmax attention,
output projection). No collectives.

Per-core per-batch dataflow (all matmuls contract over the partition dim):
  x[256,384] --DMA--> SBUF, PE-transpose -> xT[e,t]
  qT[hd,t] = Wq_cat.T @ xT   (weights stationary, 3 e-chunks accumulated)
  kT[hd,t] = Wk_cat.T @ xT
  v[t,hd]  = xT.T @ Wv_cat   -> packed per-head as [v_h | ones] (65 cols)
  per head: scoresT[t,s] = kT_h.T@qT_h ; exp(scale*scores) on ACT; causal
    mask multiply on DVE (exp-domain, zeros); av: out[s, 0:64]+denom[s] in
    one accumulating matmul with the packed [v|1] rhs; normalize with
    per-partition reciprocal scale on ACT into concat layout out[s, hd].
  PE-transpose concat -> outT[hd,s]; proj = outT.T @ Wo (+bias) -> y.
Softmax max-subtraction is skipped deliberately: scores = (q.k)/8 with
x~N(0,1), W~0.02*N(0,1) => |scores| < ~2, exp() is well-conditioned in fp32.
"""

import os
import sys

import numpy as np

sys.path.insert(0, "/opt/trn_rl_repo")

B, T, E, H, D = 256, 256, 384, 6, 64
HD = H * D  # 384
N_CORES = 8
BL = B // N_CORES  # 32 batches per core

# matmul operand dtype: "float32" (safe) or "float32r" (fast fp32 path)
MM_DT_NAME = os.environ.get("KERNEL_MM_DT", "float32r")


def _build_program(n_batches=BL, reps=1):
    import concourse.mybir as mybir
    import concourse.tile as tile
    from concourse import bacc

    FP = mybir.dt.float32
    MM = getattr(mybir.dt, MM_DT_NAME)
    AF = mybir.ActivationFunctionType

    nc = bacc.Bacc(
        "TRN2",
        target_bir_lowering=False,
        debug=False,
        enable_asserts=False,
        num_devices=N_CORES,
        enable_partition_id=False,
    )

    x_d = nc.dram_tensor("x", (n_batches * T, E), FP, kind="ExternalInput").ap()
    wq_d = nc.dram_tensor("wq", (E, HD), FP, kind="ExternalInput").ap()
    wk_d = nc.dram_tensor("wk", (E, HD), FP, kind="ExternalInput").ap()
    wv_d = nc.dram_tensor("wv", (E, HD), FP, kind="ExternalInput").ap()
    wo_d = nc.dram_tensor("wo", (HD, E), FP, kind="ExternalInput").ap()
    bo_d = nc.dram_tensor("bo", (128, E), FP, kind="ExternalInput").ap()
    mk_d = nc.dram_tensor("mask", (128, 128), FP, kind="ExternalInput").ap()
    id_d = nc.dram_tensor("ident", (128, 128), FP, kind="ExternalInput").ap()
    y_d = nc.dram_tensor("y", (n_batches * T, E), FP, kind="ExternalOutput").ap()

    def mm_ap(ap):
        return ap.bitcast(MM) if MM != FP else ap

    with tile.TileContext(nc) as tc:
        from contextlib import ExitStack

        with ExitStack() as ctx:
            const = ctx.enter_context(tc.tile_pool(name="const", bufs=1))
            wq_t = const.tile([128, 3 * HD], FP, tag="wq")
            wk_t = const.tile([128, 3 * HD], FP, tag="wk")
            wv_t = const.tile([128, 3 * HD], FP, tag="wv")
            wo_t = const.tile([128, 3 * E], FP, tag="wo")
            bo_t = const.tile([128, E], FP, tag="bo")
            mk_t = const.tile([128, 128], FP, tag="mask")
            id_t = const.tile([128, 128], FP, tag="ident")
            for t_, d_ in ((wq_t, wq_d), (wk_t, wk_d), (wv_t, wv_d), (wo_t, wo_d)):
                nc.sync.dma_start(
                    t_[:].rearrange("p (c n) -> p c n", c=3),
                    d_.rearrange("(c p) n -> p c n", p=128),
                )
            nc.sync.dma_start(bo_t[:], bo_d)
            nc.sync.dma_start(mk_t[:], mk_d)
            nc.sync.dma_start(id_t[:], id_d)

            xpool = ctx.enter_context(tc.tile_pool(name="x", bufs=2))
            xTpool = ctx.enter_context(tc.tile_pool(name="xT", bufs=2))
            qkpool = ctx.enter_context(tc.tile_pool(name="qk", bufs=2))
            vppool = ctx.enter_context(tc.tile_pool(name="vp", bufs=2))
            exppool = ctx.enter_context(tc.tile_pool(name="exp", bufs=3))
            rpool = ctx.enter_context(tc.tile_pool(name="rc", bufs=4))
            opool = ctx.enter_context(tc.tile_pool(name="oc", bufs=2))
            oTpool = ctx.enter_context(tc.tile_pool(name="oT", bufs=2))
            fpool = ctx.enter_context(tc.tile_pool(name="fin", bufs=3))

            ps_tr = ctx.enter_context(tc.tile_pool(name="ps_tr", bufs=2, space="PSUM"))
            ps_mm = ctx.enter_context(tc.tile_pool(name="ps_mm", bufs=2, space="PSUM"))
            ps_sc = ctx.enter_context(tc.tile_pool(name="ps_sc", bufs=2, space="PSUM"))
            ps_av = ctx.enter_context(tc.tile_pool(name="ps_av", bufs=2, space="PSUM"))

            def _batch_loop():
                for b in range(n_batches):
                    _one_batch(b)

            def _one_batch(b):
                # ---- load x_b and transpose to xT [e, t] ----
                x_t = xpool.tile([128, 2 * E], FP, tag="x")
                nc.sync.dma_start(
                    x_t[:].rearrange("p (c n) -> p c n", c=2),
                    x_d[b * T : (b + 1) * T, :].rearrange("(c p) n -> p c n", p=128),
                )
                xT_t = xTpool.tile([128, 3 * T], FP, tag="xT")
                for t_c in range(2):
                    for ec in range(3):
                        pt = ps_tr.tile([128, 128], FP, tag="ptr")
                        nc.tensor.transpose(
                            pt[:], x_t[:, t_c * E + ec * 128 : t_c * E + ec * 128 + 128], id_t[:]
                        )
                        nc.vector.tensor_copy(
                            xT_t[:, ec * T + t_c * 128 : ec * T + t_c * 128 + 128], pt[:]
                        )

                # ---- QKV projections ----
                qT_t = qkpool.tile([128, 3 * T], FP, tag="qT")
                kT_t = qkpool.tile([128, 3 * T], FP, tag="kT")
                for dst, w_t in ((qT_t, wq_t), (kT_t, wk_t)):
                    for hb in range(3):
                        pq = ps_sc.tile([128, T], FP, tag="psc")
                        for ec in range(3):
                            nc.tensor.matmul(
                                pq[:],
                                mm_ap(w_t[:, ec * HD + hb * 128 : ec * HD + hb * 128 + 128]),
                                mm_ap(xT_t[:, ec * T : (ec + 1) * T]),
                                start=(ec == 0),
                                stop=(ec == 2),
                            )
                        nc.vector.tensor_copy(dst[:, hb * T : (hb + 1) * T], pq[:])

                vp_t = vppool.tile([128, 2 * 390], FP, tag="vp")
                for t_c in range(2):
                    pv = ps_mm.tile([128, HD], FP, tag="pmm")
                    for ec in range(3):
                        nc.tensor.matmul(
                            pv[:],
                            mm_ap(xT_t[:, ec * T + t_c * 128 : ec * T + t_c * 128 + 128]),
                            mm_ap(wv_t[:, ec * HD : (ec + 1) * HD]),
                            start=(ec == 0),
                            stop=(ec == 2),
                        )
                    dst3 = vp_t[:, t_c * 390 : (t_c + 1) * 390].rearrange(
                        "p (h c) -> p h c", c=65
                    )
                    nc.vector.tensor_copy(
                        dst3[:, :, 0:64], pv[:].rearrange("p (h d) -> p h d", d=64)
                    )
                    nc.vector.memset(dst3[:, :, 64:65], 1.0)

                # ---- attention per head ----
                oc0 = opool.tile([128, HD], FP, tag="oc0")
                oc1 = opool.tile([128, HD], FP, tag="oc1")
                ocs = (oc0, oc1)
                for h in range(H):
                    hb, ho = divmod(h, 2)
                    po = ho * 64
                    q_all = qT_t[po : po + 64, hb * T : (hb + 1) * T]
                    exp0 = exppool.tile([128, T], FP, tag="exp0")
                    exp1 = exppool.tile([128, 128], FP, tag="exp1")

                    s0 = ps_sc.tile([128, T], FP, tag="psc")
                    nc.tensor.matmul(
                        s0[:],
                        mm_ap(kT_t[po : po + 64, hb * T : hb * T + 128]),
                        mm_ap(q_all),
                        start=True,
                        stop=True,
                    )
                    nc.scalar.activation(exp0[:], s0[:], AF.Exp, scale=0.125)
                    nc.vector.tensor_mul(exp0[:, 0:128], exp0[:, 0:128], mk_t[:])

                    s1f = ps_sc.tile([128, T], FP, tag="psc")
                    s1 = s1f[:, 0:128]
                    nc.tensor.matmul(
                        s1[:],
                        mm_ap(kT_t[po : po + 64, hb * T + 128 : hb * T + T]),
                        mm_ap(qT_t[po : po + 64, hb * T + 128 : hb * T + T]),
                        start=True,
                        stop=True,
                    )
                    nc.scalar.activation(exp1[:], s1[:], AF.Exp, scale=0.125)
                    nc.vector.tensor_mul(exp1[:], exp1[:], mk_t[:])

                    for s_c in range(2):
                        pav = ps_av.tile([128, 65], FP, tag="pav")
                        if s_c == 0:
                            nc.tensor.matmul(
                                pav[:],
                                mm_ap(exp0[:, 0:128]),
                                mm_ap(vp_t[:, h * 65 : h * 65 + 65]),
                                start=True,
                                stop=True,
                            )
                        else:
                            nc.tensor.matmul(
                                pav[:],
                                mm_ap(exp0[:, 128:256]),
                                mm_ap(vp_t[:, h * 65 : h * 65 + 65]),
                                start=True,
                                stop=False,
                            )
                            nc.tensor.matmul(
                                pav[:],
                                mm_ap(exp1[:]),
                                mm_ap(vp_t[:, 390 + h * 65 : 390 + h * 65 + 65]),
                                start=False,
                                stop=True,
                            )
                        rc = rpool.tile([128, 1], FP, tag="rc")
                        nc.vector.reciprocal(rc[:], pav[:, 64:65])
                        nc.scalar.activation(
                            ocs[s_c][:, h * 64 : (h + 1) * 64],
                            pav[:, 0:64],
                            AF.Copy,
                            scale=rc[:],
                        )

                # ---- transpose concat + output projection ----
                oT_t = oTpool.tile([128, 3 * T], FP, tag="oT")
                for s_c in range(2):
                    for hc in range(3):
                        pt = ps_tr.tile([128, 128], FP, tag="ptr")
                        nc.tensor.transpose(
                            pt[:], ocs[s_c][:, hc * 128 : (hc + 1) * 128], id_t[:]
                        )
                        nc.vector.tensor_copy(
                            oT_t[:, hc * T + s_c * 128 : hc * T + s_c * 128 + 128], pt[:]
                        )
                for s_c in range(2):
                    pp = ps_mm.tile([128, E], FP, tag="pmm")
                    for hc in range(3):
                        nc.tensor.matmul(
                            pp[:],
                            mm_ap(oT_t[:, hc * T + s_c * 128 : hc * T + s_c * 128 + 128]),
                            mm_ap(wo_t[:, hc * E : (hc + 1) * E]),
                            start=(hc == 0),
                            stop=(hc == 2),
                        )
                    fin = fpool.tile([128, E], FP, tag="fin")
                    nc.vector.tensor_add(fin[:], pp[:], bo_t[:])
                    nc.sync.dma_start(
                        y_d[b * T + s_c * 128 : b * T + s_c * 128 + 128, :], fin[:]
                    )

            if reps == 1:
                _batch_loop()
            else:
                with tc.For_i(0, reps, 1):
                    _batch_loop()

    nc.finalize()
    return nc


def _host_inputs(x, Wq, Wk, Wv, Wo, bo):
    x = np.ascontiguousarray(np.asarray(x, dtype=np.float32))
    wq = np.ascontiguousarray(
        np.asarray(Wq, dtype=np.float32).transpose(1, 0, 2).reshape(E, HD)
    )
    wk = np.ascontiguousarray(
        np.asarray(Wk, dtype=np.float32).transpose(1, 0, 2).reshape(E, HD)
    )
    wv = np.ascontiguousarray(
        np.asarray(Wv, dtype=np.float32).transpose(1, 0, 2).reshape(E, HD)
    )
    wo = np.ascontiguousarray(np.asarray(Wo, dtype=np.float32))
    bo_rep = np.ascontiguousarray(
        np.tile(np.asarray(bo, dtype=np.float32).reshape(1, E), (128, 1))
    )
    mask = np.triu(np.ones((128, 128), dtype=np.float32))
    ident = np.eye(128, dtype=np.float32)
    return x, wq, wk, wv, wo, bo_rep, mask, ident


def kernel(x, Wq, Wk, Wv, Wo, bo, _trace=False, _n_batches=BL, _reps=1):
    from concourse import bass_utils

    x, wq, wk, wv, wo, bo_rep, mask, ident = _host_inputs(x, Wq, Wk, Wv, Wo, bo)

    nc = _build_program(_n_batches, _reps)
    in_maps = []
    for c in range(N_CORES):
        xs = x[c * BL : c * BL + _n_batches].reshape(_n_batches * T, E)
        in_maps.append(
            {
                "x": np.ascontiguousarray(xs),
                "wq": wq,
                "wk": wk,
                "wv": wv,
                "wo": wo,
                "bo": bo_rep,
                "mask": mask,
                "ident": ident,
            }
        )
    res = bass_utils.run_bass_kernel_spmd(
        nc, in_maps, core_ids=list(range(N_CORES)), trace=_trace
    )
    y = np.concatenate(
        [r["y"].reshape(_n_batches, T, E) for r in res.results], axis=0
    ).astype(np.float32)
    if _trace:
        return y, res
    return y
